# revision 13
# baseline (speedup 1.0000x reference)
"""Trainium2 Bass kernel for nn_CrossModalAttention.

Problem: bidirectional cross-attention between two (B, C, H, W) feature maps.
  B=4, C=256, H=W=64 -> N=4096 pixels, HID=64.
  For each direction:  q = Wq@xq, k = Wk@xkv, v = Wv@xkv (1x1 convs),
  attn = softmax_m(q^T k), out = xq + gamma * (v @ attn^T).

Sharding: 2 directions x 4 batches = 8 independent units, one per NeuronCore.

Per-core layout: compute S^T tiles [m(part)=128, n(free)=512] via
matmul(lhsT=k_tile, rhs=q_tile) (contraction over HID=64 on partitions).
All matmul operands are bf16: fp32 operands stream through the PE at half
rate (fp32_mode=HIGH measured 501ns vs 226ns for N=512), and numpy
simulation of the full bf16 pipeline gives rel err ~5e-3 (budget 2e-2).

Key scheduling facts (measured on HW):
  - K=64 S-matmuls issued back-to-back with lhsT base partitions 0/64
    auto-derive tile_position row groups and run CONCURRENTLY (dstart
    ~3ns).  So the even/odd m-block S matmuls are emitted adjacently.
  - The PE queue is in-order: PV (which waits on exp) must not sit
    between S matmuls, so S(j+1) is issued before PV(j) (software
    pipelining) to keep the PE streaming.
  - ACT costs (N+352)/1.2 ns per ACTIVATE: exp is done on [128,1024]
    2-PSUM-bank pair tiles to amortize the fixed overhead.
  - HAM clock: the PE runs at 1.2 GHz until ~3.4us of sustained busy;
    a warm-up burst of small matmuls runs during the initial DMAs.

The v projection is folded AFTER the attention sum via associativity:
U = Wv @ (X2 @ E) -- saves nothing in FLOPs but avoids materializing V
and lets X2^T (host-transposed, bf16) be the PV weight operand.

Softmax algebra: per-n additive logit terms cancel between numerator and
denominator, so q/k biases reduce to a per-m term a_m = bq.(Wk xkv)_m,
handled (only when nonzero; setup_inputs uses zero biases) by per-half
exp bias tiles and host-scaled x2tb rows.  gamma and bv are host-folded
(gbv = gamma*bv, gam = gamma broadcast).

Denominator: running bf16 sums of exp pair-tiles on DVE (2 groups of 8
pairs per n-tile), reduced across partitions by ones-matmuls into PSUM.
Reciprocal runs on a [128,4] reshape (DRAM roundtrip) instead of [1,512]
(DVE reciprocal is 8 cycles/elem *per lane*; [1,512] wastes 127 lanes).
"""

import sys

if "/opt/trn_rl_repo" not in sys.path:
    sys.path.insert(0, "/opt/trn_rl_repo")

import ml_dtypes
import numpy as np

B = 4
C = 256
HID = 64
N = 4096          # H*W
P = 128           # SBUF partitions
NT = 512          # n-tile (matmul moving free dim)
N_NT = N // NT    # 8
MB = 128          # m-block (PV contraction tile)
N_MB = N // MB    # 32
NPAIR = N_MB // 2 # 16 m-block pairs per n-tile
CA = C // P       # 2 c-chunks / c-blocks
NCH = 1024        # x DMA chunk (columns; 2KB bf16 per partition line)
DG = 8            # pair-steps per denominator group (2 groups per n-tile)
WARM_MMS = 44     # PE warm-up burst size

_CACHE = {}


def _build_program(with_qk_bias=False):
    import concourse.bass as bass
    import concourse.mybir as mybir
    from concourse import tile

    f32 = mybir.dt.float32
    bf16 = mybir.dt.bfloat16
    AF = mybir.ActivationFunctionType

    nc = bass.Bass("TRN2", target_bir_lowering=False, debug=False)

    # x inputs are host-packed to partition-major layouts so every DMA line
    # is >=2KB contiguous per partition (full HBM bandwidth):
    #   xqp/xkvp[p, a*N+n] = x[a*128+p, n],  x2p[p, mb*C+c] = x2^T[mb*128+p, c]
    xqp_d = nc.dram_tensor("xqp", (P, CA * N), bf16, kind="ExternalInput")
    xkvp_d = nc.dram_tensor("xkvp", (P, CA * N), bf16, kind="ExternalInput")
    x2p_d = nc.dram_tensor("x2p", (P, N_MB * C), bf16, kind="ExternalInput")
    wqT_d = nc.dram_tensor("wqT", (C, HID), bf16, kind="ExternalInput")
    wkT_d = nc.dram_tensor("wkT", (C, HID), bf16, kind="ExternalInput")
    wvT_d = nc.dram_tensor("wvT", (C, C), bf16, kind="ExternalInput")
    gbv_d = nc.dram_tensor("gbv", (C, 1), f32, kind="ExternalInput")   # gamma*bv
    # rones = bf16(1/gamma): dp lhsT, so rdb = 1/(rones*d) = gamma/d with the
    # bf16 rounding residual folded into wvT on the host (exact)
    rones_d = nc.dram_tensor("rones", (P, 1), bf16, kind="ExternalInput")
    if with_qk_bias:
        # a32[p, mb] = bq . k_raw[:, mb*128+p]  (exp bias, per-m term)
        a32_d = nc.dram_tensor("a32", (P, N_MB), f32, kind="ExternalInput")
    out_d = nc.dram_tensor("out", (C, N), f32, kind="ExternalOutput")

    xqb_r = xqp_d[:].rearrange("p (a n) -> p a n", a=CA)
    xkvb_r = xkvp_d[:].rearrange("p (a n) -> p a n", a=CA)
    x2t_r = x2p_d[:].rearrange("p (mb c) -> p mb c", mb=N_MB)
    wqT_r = wqT_d[:].rearrange("(a p) h -> p a h", p=P)
    wkT_r = wkT_d[:].rearrange("(a p) h -> p a h", p=P)
    wvT_r = wvT_d[:].rearrange("(a p) c -> p a c", p=P)
    gbv_r = gbv_d[:].rearrange("(a p) one -> p (a one)", p=P)
    out_r = out_d[:].rearrange("(a p) n -> p a n", p=P)

    with tile.TileContext(nc) as tc:
        with (
            tc.tile_pool(name="const", bufs=1) as const,
            tc.tile_pool(name="xin", bufs=1) as xin,
            tc.tile_pool(name="vtp", bufs=1) as vtp,
            tc.tile_pool(name="qk", bufs=1) as qk,
            tc.tile_pool(name="work", bufs=3) as work,
            tc.tile_pool(name="ep", bufs=2) as ep,
            tc.tile_pool(name="dram", bufs=2, space="DRAM") as dram,
            tc.tile_pool(name="psum", bufs=1, space="PSUM") as psum,
        ):
            # ---- constants / weights (ACT HWDGE queue) ----
            wk_sb = const.tile([P, CA, HID], bf16, tag="wk")
            nc.scalar.dma_start(wk_sb[:], wkT_r)
            wq_sb = const.tile([P, CA, HID], bf16, tag="wq")
            nc.scalar.dma_start(wq_sb[:], wqT_r)
            wv_sb = const.tile([P, CA, C], bf16, tag="wv")
            nc.scalar.dma_start(wv_sb[:], wvT_r)
            gbv_sb = const.tile([P, CA], f32, tag="gbv")
            nc.scalar.dma_start(gbv_sb[:], gbv_r)
            rones_sb = const.tile([P, 1], bf16, tag="rones")
            nc.scalar.dma_start(rones_sb[:], rones_d[:])
            if with_qk_bias:
                a32_sb = const.tile([P, N_MB], f32, tag="a32")
                nc.scalar.dma_start(a32_sb[:], a32_d[:])
            warm_sb = const.tile([P, 64], bf16, tag="warm")
            nc.vector.memset(warm_sb[:], 0.0)

            # ---- x loads, 3 parallel HWDGE queues, 2KB lines ----
            xqb_sb = xin.tile([P, CA, N], bf16, tag="xqb")
            xkvb_sb = xin.tile([P, CA, N], bf16, tag="xkvb")
            x2t_sb = vtp.tile([P, N_MB, C], bf16, tag="x2t")

            for ch in range(N // NCH):       # xkv: SP queue (gates kproj)
                sl = slice(ch * NCH, (ch + 1) * NCH)
                for a in range(CA):
                    nc.sync.dma_start(xkvb_sb[:, a, sl], xkvb_r[:, a, sl])
            for ch in range(N // NCH):       # xq: ACT queue (after weights)
                sl = slice(ch * NCH, (ch + 1) * NCH)
                for a in range(CA):
                    nc.scalar.dma_start(xqb_sb[:, a, sl], xqb_r[:, a, sl])
            for g in range(4):               # x2^T: gpsimd queue, 4KB lines
                msl = slice(g * (N_MB // 4), (g + 1) * (N_MB // 4))
                nc.gpsimd.dma_start(x2t_sb[:, msl, :], x2t_r[:, msl, :])

            # ---- PE warm-up burst (HAM un-throttle during initial DMAs) ----
            scrW = psum.tile([P, 2 * NT], f32, tag="st2", bufs=2, name="warmps")
            for i in range(WARM_MMS):
                nc.tensor.matmul(
                    scrW[0:64, 0:64], lhsT=warm_sb[:, 0:64], rhs=warm_sb[:, 0:64],
                    start=True, stop=True,
                )

            # ---- projections (all bf16) ----
            # k proj -> "scr" PSUM bank (shared with rdb: kproj only exists
            # during nt=0, rdb only from nt=1 on); q proj -> "st2" rotation.
            # The PSUM->SBUF dup copies run on DVE (q/k) with bf16 output.
            q_sb = qk.tile([P, N], bf16, tag="q")
            k_sb = qk.tile([P, N], bf16, tag="k")

            def _kproj(ch):
                sl = slice(ch * NT, (ch + 1) * NT)
                kp = psum.tile([P, NT], f32, tag="scr", bufs=1, name=f"kp_{ch}")
                for a in range(CA):
                    nc.tensor.matmul(
                        kp[:HID, :], lhsT=wk_sb[:, a, :], rhs=xkvb_sb[:, a, sl],
                        start=(a == 0), stop=(a == CA - 1),
                    )
                nc.vector.tensor_copy(k_sb[0:HID, sl], kp[:HID, :])
                nc.vector.tensor_copy(k_sb[HID:P, sl], kp[:HID, :])

            def _qproj(nt):
                sl = slice(nt * NT, (nt + 1) * NT)
                qp = psum.tile([P, 2 * NT], f32, tag="st2", bufs=2, name=f"qp_{nt}")
                for a in range(CA):
                    nc.tensor.matmul(
                        qp[:HID, 0:NT], lhsT=wq_sb[:, a, :], rhs=xqb_sb[:, a, sl],
                        start=(a == 0), stop=(a == CA - 1),
                    )
                nc.vector.tensor_copy(q_sb[0:HID, sl], qp[:HID, 0:NT])
                nc.vector.tensor_copy(q_sb[HID:P, sl], qp[:HID, 0:NT])

            _kproj(0)
            _qproj(0)

            # ---- attention ----
            def _spair(nt, j, name):
                """Adjacent even/odd S matmuls -> concurrent PE row halves."""
                ntsl = slice(nt * NT, (nt + 1) * NT)
                me = slice((2 * j) * MB, (2 * j) * MB + MB)
                mo = slice((2 * j + 1) * MB, (2 * j + 1) * MB + MB)
                sp = psum.tile([P, 2 * NT], f32, tag="st2", bufs=2, name=name)
                nc.tensor.matmul(
                    sp[:, 0:NT], lhsT=k_sb[0:HID, me], rhs=q_sb[0:HID, ntsl],
                    start=True, stop=True,
                )
                nc.tensor.matmul(
                    sp[:, NT:], lhsT=k_sb[HID:P, mo], rhs=q_sb[HID:P, ntsl],
                    start=True, stop=True,
                )
                return sp

            def _exp(sp, nt, j):
                ex = work.tile([P, 2 * NT], bf16, tag="ex", name=f"ex_{nt}_{j}")
                if with_qk_bias:
                    nc.scalar.activation(
                        ex[:, 0:NT], sp[:, 0:NT], AF.Exp,
                        bias=a32_sb[:, 2 * j : 2 * j + 1],
                    )
                    nc.scalar.activation(
                        ex[:, NT:], sp[:, NT:], AF.Exp,
                        bias=a32_sb[:, 2 * j + 1 : 2 * j + 2],
                    )
                else:
                    nc.scalar.activation(ex[:], sp[:], AF.Exp)
                return ex

            def _epilogue_a(nt, y0, y1, dp):
                """Free the y PSUM banks (bf16 copies on ACT) and launch the
                reciprocal + partition-broadcast roundtrip.  dp was built with
                a bf16(1/gamma)-valued lhsT, so 1/dp = gamma/d directly."""
                yb0 = ep.tile([P, NT], bf16, tag="yb0", name=f"yb0_{nt}")
                nc.scalar.copy(yb0[:], y0[:])
                yb1 = ep.tile([P, NT], bf16, tag="yb1", name=f"yb1_{nt}")
                nc.scalar.copy(yb1[:], y1[:])
                rds = ep.tile([1, NT], f32, tag="rds", name=f"rds_{nt}")
                nc.vector.tensor_copy(rds[:], dp[:])
                dscr = dram.tile([1, NT], f32, tag="dscr", name=f"dscr_{nt}")
                nc.gpsimd.dma_start(dscr[:], rds[:])
                dv4 = ep.tile([P, 4], f32, tag="dv4", name=f"dv4_{nt}")
                nc.gpsimd.dma_start(
                    dv4[:], dscr[:].rearrange("o (p f) -> (o p) f", p=P)
                )
                rv4 = ep.tile([P, 4], f32, tag="rv4", name=f"rv4_{nt}")
                nc.vector.reciprocal(rv4[:], dv4[:])
                dsc2 = dram.tile([1, NT], f32, tag="dsc2", name=f"dsc2_{nt}")
                nc.gpsimd.dma_start(
                    dsc2[:].rearrange("o (p f) -> (o p) f", p=P), rv4[:]
                )
                rdb = ep.tile([P, NT], f32, tag="rdb", name=f"rdb_{nt}")
                nc.gpsimd.dma_start(rdb[:], dsc2[:].broadcast_to((P, NT)))
                return yb0, yb1, rdb

            def _epilogue_b(nt, state, cb, ups2):
                """U = Wv @ Y (one c-block), out = xq + rdb*U + gamma*bv."""
                yb0, yb1, rdb = state
                ntsl = slice(nt * NT, (nt + 1) * NT)
                ups = ups2[:, cb * NT : (cb + 1) * NT]
                nc.tensor.matmul(
                    ups, lhsT=wv_sb[:, 0, cb * P : (cb + 1) * P],
                    rhs=yb0[:], start=True, stop=False,
                )
                nc.tensor.matmul(
                    ups, lhsT=wv_sb[:, 1, cb * P : (cb + 1) * P],
                    rhs=yb1[:], start=False, stop=True,
                )
                t = ep.tile([P, NT], f32, tag="t", name=f"t_{nt}_{cb}")
                nc.vector.tensor_mul(t[:], ups, rdb[:])
                o = ep.tile([P, NT], f32, tag="o", name=f"o_{nt}_{cb}")
                nc.vector.scalar_tensor_tensor(
                    o[:],
                    in0=t[:],
                    scalar=gbv_sb[:, cb : cb + 1],
                    in1=xqb_sb[:, cb, ntsl],
                    op0=mybir.AluOpType.add,
                    op1=mybir.AluOpType.add,
                )
                nc.sync.dma_start(out_r[:, cb, ntsl], o[:])

            prev = None          # (nt-1)'s (y0, y1, dp) awaiting epilogue
            state = None         # epilogue_a output for (nt-1)
            pend_dp = None       # deferred final dp matmul pair of prev nt
            sp_next = None       # software-pipelined S pair tile
            ex_next = None

            for nt in range(N_NT):
                ntsl = slice(nt * NT, (nt + 1) * NT)
                y0 = psum.tile([P, NT], f32, tag="y", bufs=2, name=f"y0_{nt}")
                y1 = psum.tile([P, NT], f32, tag="y", bufs=2, name=f"y1_{nt}")
                dp = psum.tile([1, NT], f32, tag="dd", bufs=1, name=f"dp_{nt}")

                if nt == 0:
                    sp_next = _spair(0, 0, "sp_0_0")
                    ex_next = _exp(sp_next, 0, 0)
                else:
                    # deferred final dp matmuls of prev nt, then its epilogue
                    # head: free y banks, launch the reciprocal chain
                    if pend_dp is not None:
                        pend_dp()
                        pend_dp = None
                    state = _epilogue_a(nt - 1, *prev)

                acc = None
                for j in range(NPAIR):
                    sp, ex = sp_next, ex_next
                    # next S pair (concurrent row halves), ahead of PV(j)
                    if j + 1 < NPAIR:
                        sp_next = _spair(nt, j + 1, f"sp_{nt}_{j+1}")
                    elif nt + 1 < N_NT:
                        sp_next = _spair(nt + 1, 0, f"sp_{nt+1}_0")
                    else:
                        sp_next = None
                    # PV: 4 accumulating matmuls (2 m-blocks x 2 c-blocks)
                    first, last = (j == 0), (j == NPAIR - 1)
                    nc.tensor.matmul(
                        y0[:], lhsT=x2t_sb[:, 2 * j, 0:P], rhs=ex[:, 0:NT],
                        start=first, stop=False,
                    )
                    nc.tensor.matmul(
                        y1[:], lhsT=x2t_sb[:, 2 * j, P:C], rhs=ex[:, 0:NT],
                        start=first, stop=False,
                    )
                    nc.tensor.matmul(
                        y0[:], lhsT=x2t_sb[:, 2 * j + 1, 0:P], rhs=ex[:, NT:],
                        start=False, stop=last,
                    )
                    nc.tensor.matmul(
                        y1[:], lhsT=x2t_sb[:, 2 * j + 1, P:C], rhs=ex[:, NT:],
                        start=False, stop=last,
                    )
                    if sp_next is not None:
                        ex_next = _exp(
                            sp_next,
                            nt if j + 1 < NPAIR else nt + 1,
                            (j + 1) % NPAIR,
                        )
                    # interleaved projections / k chunks / prev-nt epilogues
                    if nt == 0 and j < 14 and j % 2 == 0:
                        _kproj(j // 2 + 1)
                    if j == 1 and nt + 1 < N_NT:
                        _qproj(nt + 1)
                    if j == 8 and state is not None:
                        ups2 = psum.tile(
                            [P, 2 * NT], f32, tag="st2", bufs=2,
                            name=f"ups_{nt-1}",
                        )
                        _epilogue_b(nt - 1, state, 0, ups2)
                    if j == 10 and state is not None:
                        _epilogue_b(nt - 1, state, 1, ups2)
                        state = None
                        ups2 = None
                    # denominator: running bf16 pair-tile sums on DVE, one
                    # ones-matmul pair per DG pair-steps accumulated into dp
                    if acc is None:
                        acc = ex
                    else:
                        s_ = work.tile(
                            [P, 2 * NT], bf16, tag="dacc", bufs=2,
                            name=f"da_{nt}_{j}",
                        )
                        nc.vector.tensor_add(s_[:], acc[:], ex[:])
                        acc = s_
                    if (j + 1) % DG == 0:
                        g = j // DG
                        a8 = acc

                        def _dmm(a8=a8, g=g, dp=dp):
                            nc.tensor.matmul(
                                dp[:], lhsT=rones_sb[:], rhs=a8[:, 0:NT],
                                start=(g == 0), stop=False,
                            )
                            nc.tensor.matmul(
                                dp[:], lhsT=rones_sb[:], rhs=a8[:, NT:],
                                start=False, stop=(g == NPAIR // DG - 1),
                            )

                        if g == NPAIR // DG - 1:
                            pend_dp = _dmm   # run at next nt's first iter
                        else:
                            _dmm()
                        acc = None
                prev = (y0, y1, dp)

            # drain: last n-tile's epilogue
            if pend_dp is not None:
                pend_dp()
                pend_dp = None
            state = _epilogue_a(N_NT - 1, *prev)
            ups2 = psum.tile([P, 2 * NT], f32, tag="st2", bufs=2, name="ups_7")
            _epilogue_b(N_NT - 1, state, 0, ups2)
            _epilogue_b(N_NT - 1, state, 1, ups2)

    return nc


def _split_excess_waits(nc):
    """The pinned walrus build only encodes 1 sync-wait per instruction;
    newer concourse attaches more. Hoist excess waits onto same-engine NoOps
    inserted immediately before the over-limit instruction (semantically
    identical: same engine, same program position)."""
    import concourse.mybir as mybir
    import bass_rust

    ctr = 0
    for bbl in nc.m.functions[0].blocks:
        il = bbl.instructions
        i = 0
        while i < len(il):
            inst = il[i]
            si = inst.sync_info
            limit = 1
            if si is not None and len(si.on_wait) > limit:
                waits = list(si.on_wait)
                extra = waits[limit:]
                for j in range(0, len(extra), 1):
                    nop = mybir.InstNoOp(name=f"I-wsplit-{ctr}", ins=[], outs=[])
                    ctr += 1
                    nop.engine = inst.engine
                    nop.sync_info = bass_rust.SyncInfo(
                        on_wait=[extra[j]], on_update=[]
                    )
                    il.insert(i, nop)
                    i += 1
                si.on_wait = waits[:limit]
                inst.sync_info = si
            i += 1
    return ctr


def _get_program(with_qk_bias=False):
    key = ("nc", with_qk_bias)
    if key not in _CACHE:
        _CACHE[key] = _build_program(with_qk_bias)
    return _CACHE[key]


def _get_program_hw(with_qk_bias=False):
    """Program with the walrus sync-wait workaround applied (breaks CoreSim's
    race detector, so only applied for hardware runs)."""
    nc = _get_program(with_qk_bias)
    skey = ("split_done", with_qk_bias)
    if not _CACHE.get(skey):
        _split_excess_waits(nc)
        _CACHE[skey] = True
    return nc


def _pack_pa(x):
    """(C, N) -> (P, CA*N): row p holds [x[p, :], x[128+p, :]] contiguous."""
    return np.ascontiguousarray(
        x.reshape(CA, P, N).transpose(1, 0, 2).reshape(P, CA * N)
    )


def _make_in_maps(x1, x2, Wq, bq, Wk, bk, Wv, bv, gamma):
    g = float(np.asarray(gamma).reshape(-1)[0])
    bq = np.asarray(bq, dtype=np.float32).reshape(-1)
    bk = np.asarray(bk, dtype=np.float32).reshape(-1)
    with_qk_bias = bool(np.any(bq)) or bool(np.any(bk))
    bf = ml_dtypes.bfloat16
    # dp is accumulated with a bf16(1/gamma)-valued lhsT so that 1/dp is
    # gamma/d; the bf16 rounding residual s = g*bf16(1/g) is folded into Wv
    # (exact: Wv is bf16-rounded afterwards anyway)
    if g != 0.0:
        rg = np.float32(1.0 / g).astype(bf)
        s = np.float64(g) * np.float64(rg)
    else:
        rg, s = np.ones((), dtype=bf), 0.0
    shared = {
        "wqT": np.ascontiguousarray(Wq.T).astype(bf),
        "wkT": np.ascontiguousarray(Wk.T).astype(bf),
        "wvT": np.ascontiguousarray(s * Wv.T).astype(bf),
        "gbv": (g * np.asarray(bv, dtype=np.float32)).reshape(C, 1),
        "rones": np.full((P, 1), rg, dtype=bf),
    }
    in_maps = []
    for d in range(2):
        src_q, src_kv = (x1, x2) if d == 0 else (x2, x1)
        for b in range(B):
            xq_f = np.ascontiguousarray(src_q[b].reshape(C, N), dtype=np.float32)
            xkv_f = np.ascontiguousarray(src_kv[b].reshape(C, N), dtype=np.float32)
            x2t = xkv_f.T  # (N, C)
            m = {
                "xqp": _pack_pa(xq_f).astype(bf),
                "xkvp": _pack_pa(xkv_f).astype(bf),
                "x2p": np.ascontiguousarray(
                    x2t.reshape(N_MB, P, C).transpose(1, 0, 2).reshape(P, N_MB * C)
                ).astype(bf),
                **shared,
            }
            if with_qk_bias:
                # per-m softmax term a_m = bq.(Wk xkv)_m; the per-n terms
                # (bk.q + bq.bk) cancel between numerator and denominator
                k_raw = Wk.astype(np.float64) @ xkv_f.astype(np.float64)
                a = (bq.astype(np.float64) @ k_raw).astype(np.float32)
                m["a32"] = np.ascontiguousarray(a.reshape(N_MB, P).T)
            in_maps.append(m)
    return in_maps, with_qk_bias


def kernel(x1, x2, Wq, bq, Wk, bk, Wv, bv, gamma, _want_results=False):
    x1 = np.asarray(x1, dtype=np.float32)
    x2 = np.asarray(x2, dtype=np.float32)
    in_maps, with_qk_bias = _make_in_maps(x1, x2, Wq, bq, Wk, bk, Wv, bv, gamma)
    nc = _get_program_hw(with_qk_bias)

    from concourse.bass_utils import run_bass_kernel_spmd

    res = run_bass_kernel_spmd(nc, in_maps, core_ids=list(range(2 * B)))
    outs = [r["out"].reshape(C, 64, 64) for r in res.results]
    out1 = np.stack(outs[:B]).astype(np.float32)
    out2 = np.stack(outs[B:]).astype(np.float32)
    if _want_results:
        return (out1, out2), res
    return (out1, out2)
